# revision 30
# baseline (speedup 1.0000x reference)
"""BitNet Llama attention (B=2, S=2048, H=4096, 32 q-heads / 8 kv-heads, GQA),
distributed over 8 Trainium2 NeuronCores.

Sharding: token-sharded BitLinear QKV projections + activation quantization
(core c owns 512 consecutive global tokens), AllToAll to head-sharded
attention (core c = kv-head c + q-heads 4c..4c+3, full causal triangle —
identical instruction stream on every core, as SPMD requires), AllReduce(max)
for the o-proj activation scales, quantize, AllToAll back to token shards,
BitLinear o_proj, host concat of row slices.

v2 critical-path changes vs baseline:
- x shipped host-transposed (xT_own) so quantization happens directly in the
  [hidden, token] layout the projections need — no on-device DMA transposes.
- K/V projected first; their (small) AllToAll is issued early and hides under
  the Q projection (also absorbs SPMD arrival skew). The q AllToAll is split
  in two head-pair halves so the first half hides under the second half's
  compute.
- o-proj activation amaxes travel as one bf16 AllReduce (+ ReduceScatter for
  the own-token dequant scales); quantized o is AllToAll'd in two token-half
  chunks; the receive side uses PE transposes (not DMA transposes) to build
  the [feature, token] o_proj operand, and Wo streaming starts during
  attention.

BitLinear exactness: weights are ternarized on host and shipped as bf16
{-1,0,1}; activations are quantized on-chip to the int8 grid (magic-number
round-half-even) and stored as bf16 integers; bf16 x bf16 matmuls accumulate
exact integers in fp32 PSUM; per-token dequant scales are applied afterwards.
"""

import math
import os
import sys
from contextlib import ExitStack

import numpy as np
import ml_dtypes

for _p in ("/opt/trn_rl_repo", os.path.expanduser("~/.axon_site/_ro/trn_rl_repo")):
    if os.path.isdir(_p) and _p not in sys.path:
        sys.path.insert(0, _p)

import concourse.bass as bass
import concourse.mybir as mybir
import concourse.tile as tile
from concourse import bacc
from concourse.masks import make_identity

P = 128
H = 4096
DHEAD = 128
NH = 32
NKV = 8
NCORES = 8
MAGIC = 12582912.0  # 1.5 * 2**23: fp32 round-half-even via add/sub
LN2 = float(math.log(2.0))
INV_SQRT_D = float(np.float32(1.0) / np.float32(np.sqrt(np.float32(DHEAD))))
INV127 = float(np.float32(1.0) / np.float32(127.0))

F32 = mybir.dt.float32
BF16 = mybir.dt.bfloat16
MULT = mybir.AluOpType.mult
ADD = mybir.AluOpType.add
SUB = mybir.AluOpType.subtract
MAXOP = mybir.AluOpType.max
IDENT_FN = mybir.ActivationFunctionType.Identity
EXP_FN = mybir.ActivationFunctionType.Exp


def build_program(S=2048, B=2, collectives=True):
    """One SPMD program; per-core behavior differs only through input data."""
    T_GLOB = B * S                      # global tokens
    T_OWN = T_GLOB // NCORES            # tokens owned per core
    NT = T_OWN // P                     # own token tiles (4 at S=2048)
    QTB = S // P                        # q tiles per batch (16)
    QT_ALL = B * QTB                    # global token tiles (32)
    HT = H // P                         # hidden tiles (32)
    GF = H // NCORES                    # q-features per head group (512)
    WON = 256                           # o_proj weight slab width
    NJ = H // WON                       # o_proj output chunks

    CH_KV = P * T_OWN + NT * P * P      # [kT: 128 x T_OWN][v: NT x 128 x 128]
    CH_QH = 2 * P * T_OWN               # two head tiles [2 x 128 x T_OWN]
    CH_O = 2 * P * GF                   # two token tiles [2 x 128 x GF]

    nc = bacc.Bacc(
        "TRN2", target_bir_lowering=False, debug=False, num_devices=NCORES
    )
    groups = [list(range(NCORES))]

    x_own = nc.dram_tensor("x_own", [T_OWN, H], F32, kind="ExternalInput")
    xT_own = nc.dram_tensor("xT_own", [H, T_OWN], F32, kind="ExternalInput")
    wqT = nc.dram_tensor("wqT", [H, H], BF16, kind="ExternalInput")
    wkT = nc.dram_tensor("wkT", [H, NKV * DHEAD], BF16, kind="ExternalInput")
    wvT = nc.dram_tensor("wvT", [H, NKV * DHEAD], BF16, kind="ExternalInput")
    woT = nc.dram_tensor("woT", [H, H], BF16, kind="ExternalInput")
    scal = nc.dram_tensor("scal", [P, 8], F32, kind="ExternalInput")
    cmaskT = nc.dram_tensor("cmaskT", [P, 4 * P], BF16, kind="ExternalInput")
    onehots = nc.dram_tensor("onehots", [8, 8 * P], F32, kind="ExternalInput")
    out_own = nc.dram_tensor("out_own", [T_OWN, H], F32, kind="ExternalOutput")

    with tile.TileContext(nc) as tc, ExitStack() as ctx:
        dram = ctx.enter_context(tc.tile_pool(name="dram", bufs=1, space="DRAM"))
        const = ctx.enter_context(tc.tile_pool(name="const", bufs=1))

        kv_in = dram.tile([NCORES, CH_KV + CH_QH], BF16, allow_tmpbuf=True)
        kv_out = dram.tile([NCORES, CH_KV + CH_QH], BF16, allow_tmpbuf=True)
        qB_in = dram.tile([NCORES, CH_QH], BF16, allow_tmpbuf=True)
        qB_out = dram.tile([NCORES, CH_QH], BF16, allow_tmpbuf=True)
        oA_in = dram.tile([NCORES, CH_O], BF16, allow_tmpbuf=True)
        oA_out = dram.tile([NCORES, CH_O], BF16, allow_tmpbuf=True)
        oB_in = dram.tile([NCORES, CH_O], BF16, allow_tmpbuf=True)
        oB_out = dram.tile([NCORES, CH_O], BF16, allow_tmpbuf=True)

        ident = const.tile([P, P], BF16)
        make_identity(nc, ident)
        identf = const.tile([P, P], F32)
        make_identity(nc, identf)
        oh_sb = const.tile([8, 8 * P], F32)
        nc.sync.dma_start(oh_sb[:], onehots[:, :])
        cmask_sb = const.tile([P, 4 * P], BF16)
        nc.sync.dma_start(cmask_sb[:], cmaskT[:, :])
        scal_sb = const.tile([P, 8], F32)
        nc.sync.dma_start(scal_sb[:], scal[:, :])
        magic_col = const.tile([P, 1], F32)
        nc.vector.memset(magic_col[:], MAGIC)

        # long-lived pools, entered early for LIFO pool-stack discipline
        gw_cm = tc.tile_pool(name="gw", bufs=2)
        gw = gw_cm.__enter__()
        amx_cm = tc.tile_pool(name="amx", bufs=1)
        amx = amx_cm.__enter__()

        pxq_cm = tc.tile_pool(name="pxq", bufs=1)
        pxq = pxq_cm.__enter__()
        xqT = pxq.tile([P, HT, T_OWN], BF16)           # quantized x, transposed
        dq_cols = pxq.tile([P, NT], F32)               # amax_clip/127 per own token
        colpack = pxq.tile([P, 2 * NT], F32)           # [127/amax | amax/127] cols

        # ---- Phase A: per-token amax from natural-layout x ----
        with nc.named_scope("phA"):
            with tc.tile_pool(name="qwork", bufs=2) as qwork:
                for ti in range(NT):
                    x_t = qwork.tile([P, H], F32, tag="x")
                    nc.scalar.dma_start(x_t[:], x_own[ti * P:(ti + 1) * P, :])
                    amax = qwork.tile([P, 1], F32, tag="amax")
                    nc.vector.tensor_reduce(
                        amax[:], x_t[:], mybir.AxisListType.X, MAXOP,
                        apply_absolute_value=True,
                    )
                    amax_c = qwork.tile([P, 1], F32, tag="amaxc")
                    nc.vector.tensor_scalar(amax_c[:], amax[:], 1e-5, None, MAXOP)
                    inv = qwork.tile([P, 1], F32, tag="inv")
                    nc.vector.reciprocal(inv[:], amax_c[:])
                    nc.vector.tensor_scalar(
                        colpack[:, ti:ti + 1], inv[:], 127.0, None, MULT
                    )
                    nc.vector.tensor_scalar(
                        colpack[:, NT + ti:NT + ti + 1], amax_c[:], INV127, None, MULT
                    )
                    nc.vector.tensor_scalar(
                        dq_cols[:, ti:ti + 1], amax_c[:], INV127, None, MULT
                    )

            # ---- Phase A2: broadcast per-token rows across partitions.
            # colpack cols -> PE transpose -> [8,128] rows -> one-hot matmuls
            # replicate row ti across all 128 partitions (no partition-sliced
            # reads, which the BIR verifier forbids off 32-boundaries). ----
            a_bcast = pxq.tile([P, T_OWN], F32)        # 127/amax (x quant scale)
            bcast_q = pxq.tile([P, T_OWN], F32)
            bcast_k = pxq.tile([P, T_OWN], F32)
            with tc.tile_pool(name="bwork", bufs=1) as bwork, \
                 tc.tile_pool(name="psb", bufs=2, space="PSUM") as psb:
                pscp = psb.tile([2 * NT, P], F32, tag="cp")
                nc.tensor.transpose(pscp[:], colpack[:], identf[:])
                rows8 = bwork.tile([2 * NT, P], F32)
                nc.vector.tensor_copy(rows8[:], pscp[:])
                ps_a = psb.tile([P, T_OWN], F32, tag="b")
                ps_dq = psb.tile([P, T_OWN], F32, tag="b")
                for ti in range(NT):
                    nc.tensor.matmul(
                        ps_a[:, ti * P:(ti + 1) * P],
                        oh_sb[:, ti * P:(ti + 1) * P], rows8[:],
                        start=True, stop=True,
                    )
                    nc.tensor.matmul(
                        ps_dq[:, ti * P:(ti + 1) * P],
                        oh_sb[:, (NT + ti) * P:(NT + ti + 1) * P], rows8[:],
                        start=True, stop=True,
                    )
                nc.vector.tensor_copy(a_bcast[:], ps_a[:])
                nc.vector.tensor_scalar(
                    bcast_q[:], ps_dq[:], scal_sb[:, 0:1], INV_SQRT_D, MULT, MULT
                )
                nc.vector.tensor_scalar(
                    bcast_k[:], ps_dq[:], scal_sb[:, 1:2], None, MULT
                )

            # ---- Phase A3: quantize x directly in transposed layout ----
            with tc.tile_pool(name="qtw", bufs=4) as qtw:
                for hi in range(HT):
                    xT_t = qtw.tile([P, T_OWN], F32, tag="xT")
                    nc.sync.dma_start(xT_t[:], xT_own[hi * P:(hi + 1) * P, :])
                    t0 = qtw.tile([P, T_OWN], F32, tag="t0")
                    nc.vector.tensor_tensor(t0[:], xT_t[:], a_bcast[:], MULT)
                    t1 = qtw.tile([P, T_OWN], F32, tag="t1")
                    nc.scalar.activation(t1[:], t0[:], IDENT_FN, bias=magic_col[:])
                    nc.vector.tensor_scalar(
                        xqT[:, hi, :], t1[:], MAGIC, None, SUB
                    )

        # ---- Phase B: QKV projections (token-sharded) -> A2A chunks ----
        k_in = kv_in[:, 0:P * T_OWN].rearrange("r (p t) -> r p t", p=P)
        v_in = kv_in[:, P * T_OWN:CH_KV].rearrange("r (i p d) -> r i p d", i=NT, p=P)
        qA_in_r = kv_in[:, CH_KV:CH_KV + CH_QH].rearrange(
            "r (f p t) -> r f p t", f=2, p=P)
        qB_in_r = qB_in.rearrange("r (f p t) -> r f p t", f=2, p=P)
        wqT_r = wqT.rearrange("(hi p) o -> p hi o", p=P)
        wkT_r = wkT.rearrange("(hi p) o -> p hi o", p=P)
        wvT_r = wvT.rearrange("(hi p) o -> p hi o", p=P)
        woT_r = woT.rearrange("(hi p) o -> p hi o", p=P)

        with nc.named_scope("phB"), \
             tc.tile_pool(name="wslab", bufs=2) as wslab, \
             tc.tile_pool(name="pevac", bufs=3) as pevac, \
             tc.tile_pool(name="psp", bufs=3, space="PSUM") as psp:
            for dj in range(NKV):  # kv-head feature tiles (k proj)
                wsl = wslab.tile([P, HT, P], BF16, tag="wq", bufs=4)
                nc.scalar.dma_start(wsl[:], wkT_r[:, :, dj * P:(dj + 1) * P])
                ps = psp.tile([P, T_OWN], F32, tag="p")
                for hi in range(HT):
                    nc.tensor.matmul(
                        ps[:], wsl[:, hi, :], xqT[:, hi, :],
                        start=(hi == 0), stop=(hi == HT - 1),
                    )
                ev = pevac.tile([P, T_OWN], BF16, tag="e")
                nc.vector.tensor_tensor(ev[:], ps[:], bcast_k[:], MULT)
                nc.sync.dma_start(k_in[dj, :, :], ev[:])
            for vi in range(2):  # v natural layout, 512-wide feature chunks
                wsl = wslab.tile([P, HT, 512], BF16, tag="wv")
                nc.scalar.dma_start(wsl[:], wvT_r[:, :, vi * 512:(vi + 1) * 512])
                for ti in range(NT):
                    ps = psp.tile([P, 512], F32, tag="pv")
                    for hi in range(HT):
                        nc.tensor.matmul(
                            ps[:], xqT[:, hi, ti * P:(ti + 1) * P], wsl[:, hi, :],
                            start=(hi == 0), stop=(hi == HT - 1),
                        )
                    sv = pevac.tile([P, 1], F32, tag="sv")
                    nc.vector.tensor_scalar(
                        sv[:], dq_cols[:, ti:ti + 1], scal_sb[:, 2:3], None, MULT
                    )
                    ev = pevac.tile([P, 512], BF16, tag="ev")
                    nc.scalar.mul(ev[:], ps[:], sv[:])
                    for sub in range(4):
                        nc.sync.dma_start(
                            v_in[vi * 4 + sub, ti, :, :],
                            ev[:, sub * P:(sub + 1) * P],
                        )
            for g in range(2):  # q head-pair halves
                q_dst = qA_in_r if g == 0 else qB_in_r
                for r in range(NCORES):
                    for f in range(2):
                        dj = 4 * r + 2 * g + f
                        wsl = wslab.tile([P, HT, P], BF16, tag="wq", bufs=4)
                        nc.scalar.dma_start(wsl[:], wqT_r[:, :, dj * P:(dj + 1) * P])
                        ps = psp.tile([P, T_OWN], F32, tag="p")
                        for hi in range(HT):
                            nc.tensor.matmul(
                                ps[:], wsl[:, hi, :], xqT[:, hi, :],
                                start=(hi == 0), stop=(hi == HT - 1),
                            )
                        ev = pevac.tile([P, T_OWN], BF16, tag="e")
                        nc.vector.tensor_tensor(ev[:], ps[:], bcast_q[:], MULT)
                        nc.sync.dma_start(q_dst[r, f, :, :], ev[:])
                src_buf = kv_in if g == 0 else qB_in
                dst_buf = kv_out if g == 0 else qB_out
                if collectives:
                    nc.gpsimd.collective_compute(
                        "AllToAll", mybir.AluOpType.bypass, replica_groups=groups,
                        ins=[src_buf[:, :].opt()], outs=[dst_buf[:, :].opt()],
                    )
                else:
                    nc.sync.dma_start(dst_buf[:, :], src_buf[:, :])

        pxq_cm.__exit__(None, None, None)

        # ---- Phase D: assemble head-sharded attention operands ----
        k_out = kv_out[:, 0:P * T_OWN].rearrange("r (p t) -> r p t", p=P)
        v_out = kv_out[:, P * T_OWN:CH_KV].rearrange(
            "r (i p d) -> r p i d", i=NT, p=P
        )
        qA_out_r = kv_out[:, CH_KV:CH_KV + CH_QH].rearrange(
            "r (f p t) -> r p f t", f=2, p=P)
        qB_out_r = qB_out.rearrange("r (f p t) -> r p f t", f=2, p=P)
        pat_cm = tc.tile_pool(name="pat", bufs=1)
        pat = pat_cm.__enter__()
        qT_grp = pat.tile([P, 4, T_GLOB], BF16)
        kT_full = pat.tile([P, T_GLOB], BF16)
        v_full = pat.tile([P, QT_ALL, 132], BF16)
        nc.vector.memset(v_full[:], 1.0)  # column 128 = denominator ones
        with nc.named_scope("phD"):
            for s in range(NCORES):
                nc.sync.dma_start(
                    kT_full[:, s * T_OWN:(s + 1) * T_OWN], k_out[s, :, :]
                )
                nc.sync.dma_start(
                    v_full[:, s * NT:(s + 1) * NT, 0:P], v_out[s, :, :, :]
                )
                nc.sync.dma_start(
                    qT_grp[:, 0:2, s * T_OWN:(s + 1) * T_OWN], qA_out_r[s, :, :, :]
                )
                nc.sync.dma_start(
                    qT_grp[:, 2:4, s * T_OWN:(s + 1) * T_OWN], qB_out_r[s, :, :, :]
                )

        # prefetch first o_proj weight slab during attention
        wo_slabs = {}
        wo_slabs[0] = gw.tile([P, HT, WON], BF16, tag="wo", name="wo_slab0")
        nc.sync.dma_start(wo_slabs[0][:], woT_r[:, :, 0:WON])

        # ---- Phase E: attention (full causal triangle, 4 heads) ----
        pos_cm = tc.tile_pool(name="pos", bufs=1)
        pos = pos_cm.__enter__()
        o_slice = pos.tile([P, QT_ALL, GF], BF16)
        oA_in_r = oA_in.rearrange("r (i p f) -> r i p f", i=2, p=P)
        oB_in_r = oB_in.rearrange("r (i p f) -> r i p f", i=2, p=P)
        with nc.named_scope("attn"), \
             tc.tile_pool(name="att", bufs=4) as att, \
             tc.tile_pool(name="pss", bufs=3, space="PSUM") as pss, \
             tc.tile_pool(name="pso", bufs=4, space="PSUM") as pso:
            for b in range(B):
                for qb in range(QTB):
                    qt = b * QTB + qb
                    po = [pso.tile([P, 132], F32, tag="o", name=f"po{_h}",
                                   bufs=4)
                          for _h in range(4)]
                    pt_all = att.tile([P, QTB, 4 * P], BF16, tag="pt", bufs=3)
                    for j in range(qb + 1):
                        kt = b * QTB + j
                        ps = pss.tile([P, 4 * P], F32, tag="s")
                        nc.tensor.matmul(
                            ps[:],
                            kT_full[:, kt * P:(kt + 1) * P],
                            qT_grp[:, :, qt * P:(qt + 1) * P],
                            start=True, stop=True,
                        )
                        nc.scalar.activation(
                            pt_all[:, j, :], ps[:], EXP_FN, scale=LN2,
                        )
                        if j == qb:
                            nc.vector.tensor_tensor(
                                pt_all[:, j, :], pt_all[:, j, :],
                                cmask_sb[:], MULT,
                            )
                        for hl in range(4):
                            nc.tensor.matmul(
                                po[hl][:, 0:129],
                                pt_all[:, j, hl * P:(hl + 1) * P],
                                v_full[:, kt, 0:129],
                                start=(j == 0), stop=(j == qb),
                            )
                    for hl in range(4):
                        den = att.tile([P, 1], F32, tag="den")
                        nc.vector.reciprocal(den[:], po[hl][:, 128:129])
                        nc.vector.tensor_scalar(
                            o_slice[:, qt, hl * P:(hl + 1) * P],
                            po[hl][:, 0:P], den[:], None, MULT,
                        )
                    # ship raw bf16 o straight into the A2A staging buffer;
                    # chunk A (local token tiles {0,1}) completes at qt==29, so
                    # its AllToAll runs under the last two attention tiles.
                    r_t, lt = qt // 4, qt % 4
                    o_dst = oA_in_r if lt < 2 else oB_in_r
                    nc.sync.dma_start(o_dst[r_t, lt % 2, :, :], o_slice[:, qt, :])
                    if qt == 29 and collectives:
                        nc.gpsimd.collective_compute(
                            "AllToAll", mybir.AluOpType.bypass,
                            replica_groups=groups,
                            ins=[oA_in[:, :].opt()], outs=[oA_out[:, :].opt()],
                        )

        # ---- Phase E3: second raw-o AllToAll (chunk A went mid-attention) ----
        with nc.named_scope("oship"):
            if collectives:
                nc.gpsimd.collective_compute(
                    "AllToAll", mybir.AluOpType.bypass, replica_groups=groups,
                    ins=[oB_in[:, :].opt()], outs=[oB_out[:, :].opt()],
                )
            else:
                nc.sync.dma_start(oA_out[:, :], oA_in[:, :])
                nc.sync.dma_start(oB_out[:, :], oB_in[:, :])

        pos_cm.__exit__(None, None, None)
        pat_cm.__exit__(None, None, None)

        # ---- Phase F: assemble raw o, local per-token amax (free-dim reduce:
        # each core now holds all 4096 features of its own tokens), quantize,
        # PE-transpose into [feature, token] for o_proj. No collectives. ----
        oA_out_r = oA_out.rearrange("r (i p f) -> r i p f", i=2, p=P)
        oB_out_r = oB_out.rearrange("r (i p f) -> r i p f", i=2, p=P)
        pxo_cm = tc.tile_pool(name="pxo", bufs=1)
        pxo = pxo_cm.__enter__()
        xoqT = pxo.tile([P, HT, T_OWN], BF16)
        o_recv = pxo.tile([P, NT, H], BF16)
        dqo_cols = amx.tile([P, NT], F32)
        with nc.named_scope("phF"), \
             tc.tile_pool(name="fw", bufs=3) as fw, \
             tc.tile_pool(name="pst", bufs=4, space="PSUM") as pstp:
            for half in range(2):
                src = oA_out_r if half == 0 else oB_out_r
                for t2 in range(2):
                    ti = 2 * half + t2
                    for s in range(NCORES):
                        nc.sync.dma_start(
                            o_recv[:, ti, s * GF:(s + 1) * GF], src[s, t2, :, :]
                        )
                    am = fw.tile([P, 1], F32, tag="am")
                    nc.vector.tensor_reduce(
                        am[:], o_recv[:, ti, :], mybir.AxisListType.X, MAXOP,
                        apply_absolute_value=True,
                    )
                    amc = fw.tile([P, 1], F32, tag="amc")
                    nc.vector.tensor_scalar(amc[:], am[:], 1e-5, None, MAXOP)
                    inv = fw.tile([P, 1], F32, tag="inv")
                    nc.vector.reciprocal(inv[:], amc[:])
                    acol = fw.tile([P, 1], F32, tag="acol")
                    nc.vector.tensor_scalar(acol[:], inv[:], 127.0, None, MULT)
                    nc.vector.tensor_scalar(
                        dqo_cols[:, ti:ti + 1], amc[:], scal_sb[:, 3:4],
                        INV127, MULT, MULT,
                    )
                    for s in range(NCORES):
                        t0 = fw.tile([P, GF], F32, tag="t0")
                        nc.scalar.activation(
                            t0[:], o_recv[:, ti, s * GF:(s + 1) * GF], IDENT_FN,
                            bias=magic_col[:], scale=acol[:],
                        )
                        xq = fw.tile([P, GF], BF16, tag="xq")
                        nc.vector.tensor_scalar(xq[:], t0[:], MAGIC, None, SUB)
                        for fi in range(4):
                            pt = pstp.tile([P, P], BF16, tag="tp")
                            nc.tensor.transpose(
                                pt[:], xq[:, fi * P:(fi + 1) * P], ident[:]
                            )
                            dst_ap = xoqT[:, 4 * s + fi, ti * P:(ti + 1) * P]
                            if fi % 2 == 0:
                                nc.vector.tensor_copy(dst_ap, pt[:])
                            else:
                                nc.scalar.copy(dst_ap, pt[:])

        # ---- Phase G: o_proj (token-sharded, full output features) ----
        with nc.named_scope("phG"), \
             tc.tile_pool(name="gev", bufs=3) as gev, \
             tc.tile_pool(name="psg", bufs=3, space="PSUM") as psg:
            for nj in range(NJ):
                if nj not in wo_slabs:
                    wo_slabs[nj] = gw.tile([P, HT, WON], BF16, tag="wo", name=f"wo_slab{nj}")
                    nc.sync.dma_start(
                        wo_slabs[nj][:], woT_r[:, :, nj * WON:(nj + 1) * WON]
                    )
                wsl = wo_slabs[nj]
                for ti in range(NT):
                    ps = psg.tile([P, WON], F32, tag="g")
                    for hi in range(HT):
                        nc.tensor.matmul(
                            ps[:], xoqT[:, hi, ti * P:(ti + 1) * P], wsl[:, hi, :],
                            start=(hi == 0), stop=(hi == HT - 1),
                        )
                    ev = gev.tile([P, WON], F32, tag="ge")
                    nc.scalar.mul(ev[:], ps[:], dqo_cols[:, ti:ti + 1])
                    nc.sync.dma_start(
                        out_own[ti * P:(ti + 1) * P, nj * WON:(nj + 1) * WON], ev[:]
                    )

        pxo_cm.__exit__(None, None, None)
        amx_cm.__exit__(None, None, None)
        gw_cm.__exit__(None, None, None)

    nc.compile()
    return nc


def _ternarize(W):
    ws = np.float32(max(np.mean(np.abs(W), dtype=np.float32), np.float32(1e-5)))
    t = np.clip(np.round(W / ws), -1.0, 1.0).astype(np.float32)
    return t, ws


def prepare_inputs(hidden_states, Wq, Wk, Wv, Wo, S=2048, B=2):
    bf16 = ml_dtypes.bfloat16
    T_GLOB = B * S
    T_OWN = T_GLOB // NCORES
    x = np.ascontiguousarray(
        np.asarray(hidden_states, dtype=np.float32).reshape(T_GLOB, H)
    )
    tq, wqs = _ternarize(np.asarray(Wq, dtype=np.float32))
    tk, wks = _ternarize(np.asarray(Wk, dtype=np.float32))
    tv, wvs = _ternarize(np.asarray(Wv, dtype=np.float32))
    to, wos = _ternarize(np.asarray(Wo, dtype=np.float32))
    wqT = np.ascontiguousarray(tq.T).astype(bf16)
    wkT = np.ascontiguousarray(tk.T).astype(bf16)
    wvT = np.ascontiguousarray(tv.T).astype(bf16)
    woT = np.ascontiguousarray(to.T).astype(bf16)
    scal = np.zeros((P, 8), np.float32)
    scal[:, 0] = wqs
    scal[:, 1] = wks
    scal[:, 2] = wvs
    scal[:, 3] = wos
    kk, qq = np.meshgrid(np.arange(P), np.arange(P), indexing="ij")
    cmaskT = np.tile((kk <= qq).astype(np.float32).astype(bf16), (1, 4))
    onehots = np.zeros((8, 8 * P), np.float32)
    for j in range(8):
        onehots[j, j * P:(j + 1) * P] = 1.0
    shared = dict(wqT=wqT, wkT=wkT, wvT=wvT, woT=woT, scal=scal, cmaskT=cmaskT,
                  onehots=onehots)
    return [
        dict(
            x_own=np.ascontiguousarray(x[c * T_OWN:(c + 1) * T_OWN]),
            xT_own=np.ascontiguousarray(x[c * T_OWN:(c + 1) * T_OWN].T),
            **shared,
        )
        for c in range(NCORES)
    ]


_PROGRAM_CACHE = {}


def kernel(hidden_states, attention_mask, Wq, Wk, Wv, Wo):
    from concourse.bass_utils import run_bass_kernel_spmd

    B, S, _ = hidden_states.shape
    key = (B, S)
    if key not in _PROGRAM_CACHE:
        _PROGRAM_CACHE[key] = build_program(S=S, B=B)
    nc = _PROGRAM_CACHE[key]
    in_maps = prepare_inputs(hidden_states, Wq, Wk, Wv, Wo, S=S, B=B)
    res = run_bass_kernel_spmd(
        nc, in_maps, core_ids=list(range(NCORES)),
        trace=bool(int(os.environ.get("KERNEL_TRACE", "0"))),
    )
    out = np.concatenate([r["out_own"] for r in res.results], axis=0)
    kernel.last_results = res
    return np.ascontiguousarray(out.reshape(B, S, H)).astype(np.float32)
